# revision 7
# baseline (speedup 1.0000x reference)
"""DirGNN (3x DirGNNConv + global mean pool) on 8 Trainium2 NeuronCores.

Strategy
--------
Node-sharded data parallelism: core k owns nodes [12500k, 12500(k+1)).
Each DirGNNConv layer l and direction d is factored as

    table_{l,d} = dinv_d * (h_{l-1} @ (alpha_d * W_{l,d}))          # per-node, dense
    agg_{l,d}[head] = sum_{edges e with head(e)=head} table_{l,d}[tail(e)]
    h_l = relu(dinv_f*agg_f + dinv_r*agg_r + 0.5*(b_f + b_r))

so all per-edge work is gather + segment-sum.  Tables are built per-shard and
AllGathered (f32) between layers.  Gathers use the MoE dma_gather DMA
(int16 indices -> table is laid out as 8 x [12500-node shard + zero row] so
every 32K-row index window contains a zero row for padding).  Heads are
decomposed into power-of-two parts (per index-window) so segment sums become
uniform strided DVE reduces; partial sums are combined by dma_scatter_add
(CCE f32 accumulate) into node-ordered aggregate buffers.  The final layer
pools via a PE matmul with a host-built one-hot graph-membership matrix and
applies W_out to the pooled [*,128] vectors; the host sums the 8 partial
[120, 64] outputs, divides by graph sizes and adds biases.

The Bass program is identical on all 8 cores (SPMD); per-core structure is
padded to a common per-(window, part-size) region grid, with padding slots
gathering a zero table row and scattering into a trash row.
"""

import sys

sys.path.insert(0, "/opt/trn_rl_repo")

import numpy as np

N, E, G = 100000, 1600000, 256
IN_C, HID, OUT_C = 4, 128, 120
ALPHA = 0.5
NCORES = 8
SHARD = N // NCORES            # 12500
SHARD_PAD = 12544              # 98 * 128
NCHUNK = SHARD_PAD // 128      # 98
TROWS = SHARD + 1              # table shard rows (12500 + zero row)
WINDOW = 2 * TROWS             # 25002; int16 windows: idx in [0, 32767]
ZLOC = SHARD                   # zero row local index (=12500) in every window
AGG_ROWS = SHARD_PAD + 1       # 12545, trash row at 12544
TRASH = SHARD_PAD              # 12544
GSLOT = 64                     # max graphs per core in pooled output
KSET = (32, 16, 8, 4, 2, 1)
LAST = {}
CALL_CHUNKS = 32               # gather call size: 32*128 rows


def _table_row(n):
    return n + n // SHARD


def _build_direction(heads, tails):
    """Vectorized edge layout for one direction.

    Returns (common, per_core) where common = list of regions
    (w, k, rows) in program order plus derived call list, and per_core =
    dict core -> (gather_idx int16 [L], scatter_idx int16 [R*128]).
    """
    E_ = heads.shape[0]
    trow = _table_row(tails)
    w = trow // WINDOW
    gval = (trow - w * WINDOW).astype(np.int64)
    core = heads // SHARD
    hloc = heads % SHARD

    korder_of_k = {k: i for i, k in enumerate(KSET)}

    # group edges by (head, window); rank within group
    key = heads.astype(np.int64) * 4 + w
    order0 = np.argsort(key, kind="stable")
    ks = key[order0]
    grp_start = np.r_[True, ks[1:] != ks[:-1]]
    grp_id = np.cumsum(grp_start) - 1
    first_pos = np.flatnonzero(grp_start)
    r = np.arange(E_) - first_pos[grp_id]
    gsz = np.bincount(grp_id)
    c = gsz[grp_id]

    # power-of-two part decomposition with 32-cap
    a32 = c >> 5
    in32 = r < (a32 << 5)
    r2 = r - (a32 << 5)
    c2 = c & 31
    t16 = ((c2 >> 4) & 1) * 16
    t8 = t16 + ((c2 >> 3) & 1) * 8
    t4 = t8 + ((c2 >> 2) & 1) * 4
    t2 = t4 + ((c2 >> 1) & 1) * 2
    k_arr = np.select(
        [in32, r2 < t16, r2 < t8, r2 < t4, r2 < t2],
        [32, 16, 8, 4, 2],
        default=1,
    )
    j_arr = np.select(
        [in32, r2 < t16, r2 < t8, r2 < t4, r2 < t2],
        [r & 31, r2 - 0, r2 - t16, r2 - t8, r2 - t4],
        default=r2 - t2,
    )
    part = np.where(in32, r >> 5, 1 << 20)  # distinct parts only for k=32
    ko = np.select([k_arr == kk for kk in KSET], list(range(len(KSET))))

    # order edges by (core, w, ko, hloc, part, j) -- back-to-front keys
    eo = np.lexsort((j_arr, part, hloc[order0], ko, w[order0], core[order0]))
    so = order0[eo]  # original edge ids in final order
    core_s = core[so]
    w_s = w[so]
    ko_s = ko[eo]
    k_s = k_arr[eo]
    j_s = j_arr[eo]
    hloc_s = hloc[so]
    part_s = part[eo]
    gval_s = gval[so]

    # vdst boundaries in the sorted stream
    vkey = np.stack([core_s, w_s, ko_s, hloc_s, part_s])
    vchg = np.r_[True, (np.diff(vkey, axis=1) != 0).any(axis=0)]
    vid = np.cumsum(vchg) - 1                      # global vdst id
    n_vd = vid[-1] + 1
    vd_core = core_s[vchg]
    vd_w = w_s[vchg]
    vd_ko = ko_s[vchg]
    vd_hloc = hloc_s[vchg]

    # vdst counts per (core, w, ko)
    reg_of_vd = (vd_core * 4 + vd_w) * len(KSET) + vd_ko
    nreg = NCORES * 4 * len(KSET)
    vd_cnt = np.bincount(reg_of_vd, minlength=nreg).reshape(NCORES, 4, len(KSET))
    # common grid: rows per (w, ko) = ceil(max over cores / 128)
    rows_wk = (vd_cnt.max(axis=0) + 127) // 128    # [4, len(KSET)]

    # region order: (w, ko); region chunk offsets
    regions = []   # (w, k, rows, chunk_off, row_off)
    ch_off = 0
    row_off = 0
    for wv in range(4):
        for koi, kk in enumerate(KSET):
            rr = int(rows_wk[wv, koi])
            if rr == 0:
                continue
            regions.append((wv, kk, rr, ch_off, row_off))
            ch_off += rr * kk
            row_off += rr
    L = ch_off * 128          # total gather positions per core
    R = row_off               # total vdst rows per core

    # per-region chunk/row offset lookup tables
    chunk_off_wk = np.zeros((4, len(KSET)), np.int64)
    row_off_wk = np.zeros((4, len(KSET)), np.int64)
    for (wv, kk, rr, co, ro) in regions:
        chunk_off_wk[wv, korder_of_k[kk]] = co
        row_off_wk[wv, korder_of_k[kk]] = ro

    # vdst local index within its (core, region)
    reg_sorted = np.argsort(reg_of_vd, kind="stable")
    vd_local = np.empty(n_vd, np.int64)
    cnts = np.bincount(reg_of_vd, minlength=nreg)
    starts = np.concatenate([[0], np.cumsum(cnts)[:-1]])
    vd_local[reg_sorted] = np.arange(n_vd) - starts[reg_of_vd[reg_sorted]]

    per_core = []
    vd_local_of_edge = vd_local[vid]
    edge_pos = (
        (chunk_off_wk[w_s, ko_s] + (vd_local_of_edge // 128) * k_s + j_s) * 128
        + (vd_local_of_edge % 128)
    )
    vd_rowslot = row_off_wk[vd_w, vd_ko] * 128 + (vd_local // 128) * 128 + (vd_local % 128)

    for cc in range(NCORES):
        gidx = np.full(L, ZLOC, np.int16)
        m = core_s == cc
        gidx[edge_pos[m]] = gval_s[m]
        sidx = np.full(R * 128, TRASH, np.int16)
        mv = vd_core == cc
        sidx[vd_rowslot[mv]] = vd_hloc[mv]
        per_core.append((gidx, sidx))

    # call list (common): per region, rows per call = max(1, 32 // k)
    calls = []   # (w, k, chunk_off, rows, row_off)
    for (wv, kk, rr, co, ro) in regions:
        rpc = max(1, CALL_CHUNKS // kk)
        s = 0
        while s < rr:
            nr = min(rpc, rr - s)
            calls.append((wv, kk, co + s * kk, nr, ro + s))
            s += nr

    common = dict(regions=regions, calls=calls, L=L, R=R)
    return common, per_core


def _wrap_idx(a):
    """int16 [M] -> [128, M//16] wrapped (pos i = arr[i%16, i//16]) and
    replicated across the 8 Q7 core partition groups."""
    a16 = a.reshape(-1, 16).T
    return np.ascontiguousarray(np.tile(a16, (8, 1)), dtype=np.int16)


def kernel(x, edge_index, batch_seg, W_in, b_in, W_in_r, b_in_r,
           W_mid, b_mid, W_mid_r, b_mid_r, W_out, b_out, W_out_r, b_out_r):
    import concourse.bass as bass
    import concourse.bacc as bacc
    import concourse.mybir as mybir
    import concourse.tile as tile
    from concourse import bass_utils
    from concourse.masks import make_identity

    x = np.asarray(x, np.float32)
    ei = np.asarray(edge_index, np.int64)
    seg = np.asarray(batch_seg, np.int64)
    Wf1, Wr1 = np.asarray(W_in, np.float32), np.asarray(W_in_r, np.float32)
    Wf2, Wr2 = np.asarray(W_mid, np.float32), np.asarray(W_mid_r, np.float32)
    Wf3, Wr3 = np.asarray(W_out, np.float32), np.asarray(W_out_r, np.float32)
    src, dst = ei[0], ei[1]

    # degree-based norms (match reference: rsqrt(max(deg,1e-12)), 0 for deg==0)
    deg_f = np.bincount(dst, minlength=N).astype(np.float32)
    deg_r = np.bincount(src, minlength=N).astype(np.float32)
    dinv_f = np.where(deg_f > 0, 1.0 / np.sqrt(np.maximum(deg_f, 1e-12)), 0.0).astype(np.float32)
    dinv_r = np.where(deg_r > 0, 1.0 / np.sqrt(np.maximum(deg_r, 1e-12)), 0.0).astype(np.float32)

    com_f, pc_f = _build_direction(dst, src)     # fwd: head=dst, tail=src
    com_r, pc_r = _build_direction(src, dst)     # rev: head=src, tail=dst

    # ---------- per-core host tensors ----------
    halfa = np.float32(1.0 - ALPHA)   # fwd scale
    halfb = np.float32(ALPHA)         # rev scale
    W1f_s = (Wf1 * halfa).astype(np.float32)         # [4,128]
    W1r_s = (Wr1 * halfb).astype(np.float32)
    W2f_s = (Wf2 * halfa).astype(np.float32)         # [128,128]
    W2r_s = (Wr2 * halfb).astype(np.float32)
    W3f_s = (Wf3 * halfa).astype(np.float32)         # [128,120]
    W3r_s = (Wr3 * halfb).astype(np.float32)
    bias1 = (halfa * np.asarray(b_in, np.float32) + halfb * np.asarray(b_in_r, np.float32))
    bias2 = (halfa * np.asarray(b_mid, np.float32) + halfb * np.asarray(b_mid_r, np.float32))

    xfull_pad = np.zeros((NCORES, SHARD_PAD, IN_C), np.float32)
    seg_pad = np.zeros((NCORES, SHARD_PAD), np.int64)
    dinvf_pad = np.zeros((NCORES, SHARD_PAD), np.float32)
    dinvr_pad = np.zeros((NCORES, SHARD_PAD), np.float32)
    for cc in range(NCORES):
        xfull_pad[cc, :SHARD] = x[cc * SHARD:(cc + 1) * SHARD]
        seg_pad[cc, :SHARD] = seg[cc * SHARD:(cc + 1) * SHARD]
        dinvf_pad[cc, :SHARD] = dinv_f[cc * SHARD:(cc + 1) * SHARD]
        dinvr_pad[cc, :SHARD] = dinv_r[cc * SHARD:(cc + 1) * SHARD]

    g0s = [int(seg[cc * SHARD]) for cc in range(NCORES)]
    for cc in range(NCORES):
        gmax = int(seg_pad[cc, :SHARD].max())
        assert gmax - g0s[cc] < GSLOT, "graph range exceeds GSLOT"

    LF, RF = com_f["L"], com_f["R"]
    LR, RR = com_r["L"], com_r["R"]

    in_maps = []
    for cc in range(NCORES):
        gi_f, si_f = pc_f[cc]
        gi_r, si_r = pc_r[cc]
        # p-major [128, NCHUNK] node-chunk tiles
        dvf_t = dinvf_pad[cc].reshape(NCHUNK, 128).T.copy()
        dvr_t = dinvr_pad[cc].reshape(NCHUNK, 128).T.copy()
        # graph one-hot [128, NCHUNK*GSLOT]
        gm = np.zeros((128, NCHUNK, GSLOT), np.float32)
        locg = seg_pad[cc] - g0s[cc]
        pp = np.arange(SHARD_PAD) % 128
        ch = np.arange(SHARD_PAD) // 128
        real = np.arange(SHARD_PAD) < SHARD
        gm[pp[real], ch[real], locg[real]] = 1.0
        im = {
            "xT": np.ascontiguousarray(xfull_pad[cc].T),          # [4, SHARD_PAD]
            "dinvf": dvf_t, "dinvr": dvr_t,
            "gmat": gm.reshape(128, NCHUNK * GSLOT),
            "w1f": W1f_s, "w1r": W1r_s,
            "w2f": W2f_s, "w2r": W2r_s,
            "w3f": W3f_s, "w3r": W3r_s,
            "bias1": np.tile(bias1.reshape(1, HID), (128, 1)), "bias2": np.tile(bias2.reshape(1, HID), (128, 1)),
            "gif": _wrap_idx(gi_f), "gir": _wrap_idx(gi_r),
            "sif": _wrap_idx(si_f), "sir": _wrap_idx(si_r),
        }
        for l in (1, 2, 3):
            for d in ("f", "r"):
                im[f"agg{l}{d}"] = np.zeros((AGG_ROWS, HID), np.float32)
        in_maps.append(im)

    # ---------- bass program (identical on all cores) ----------
    nc = bacc.Bacc("TRN2", target_bir_lowering=False, debug=False,
                   enable_asserts=False, num_devices=NCORES)
    f32 = mybir.dt.float32
    i16 = mybir.dt.int16

    d_xT = nc.dram_tensor("xT", [IN_C, SHARD_PAD], f32, kind="ExternalInput").ap()
    d_dinvf = nc.dram_tensor("dinvf", [128, NCHUNK], f32, kind="ExternalInput").ap()
    d_dinvr = nc.dram_tensor("dinvr", [128, NCHUNK], f32, kind="ExternalInput").ap()
    d_gmat = nc.dram_tensor("gmat", [128, NCHUNK * GSLOT], f32, kind="ExternalInput").ap()
    d_w = {}
    d_w["w1f"] = nc.dram_tensor("w1f", [IN_C, HID], f32, kind="ExternalInput").ap()
    d_w["w1r"] = nc.dram_tensor("w1r", [IN_C, HID], f32, kind="ExternalInput").ap()
    d_w["w2f"] = nc.dram_tensor("w2f", [HID, HID], f32, kind="ExternalInput").ap()
    d_w["w2r"] = nc.dram_tensor("w2r", [HID, HID], f32, kind="ExternalInput").ap()
    d_w["w3f"] = nc.dram_tensor("w3f", [HID, OUT_C], f32, kind="ExternalInput").ap()
    d_w["w3r"] = nc.dram_tensor("w3r", [HID, OUT_C], f32, kind="ExternalInput").ap()
    d_b1 = nc.dram_tensor("bias1", [128, HID], f32, kind="ExternalInput").ap()
    d_b2 = nc.dram_tensor("bias2", [128, HID], f32, kind="ExternalInput").ap()
    d_gif = nc.dram_tensor("gif", [128, LF // 16], i16, kind="ExternalInput").ap()
    d_gir = nc.dram_tensor("gir", [128, LR // 16], i16, kind="ExternalInput").ap()
    d_sif = nc.dram_tensor("sif", [128, RF * 8], i16, kind="ExternalInput").ap()
    d_sir = nc.dram_tensor("sir", [128, RR * 8], i16, kind="ExternalInput").ap()
    d_agg = {}
    for l in (1, 2, 3):
        for d in ("f", "r"):
            d_agg[(l, d)] = nc.dram_tensor(f"agg{l}{d}", [AGG_ROWS, HID], f32,
                                           kind="ExternalInput").ap()
    d_out = nc.dram_tensor("outp", [OUT_C, GSLOT], f32, kind="ExternalOutput").ap()

    # internal HBM: table shards + gathered tables
    d_sh = {}
    d_tab = {}
    for l in (1, 2, 3):
        for d in ("f", "r"):
            d_sh[(l, d)] = nc.dram_tensor(f"sh{l}{d}", [TROWS, HID], f32, kind="Internal").ap()
            d_tab[(l, d)] = nc.dram_tensor(f"tab{l}{d}", [NCORES * TROWS, HID], f32,
                                           kind="Internal", addr_space="Shared").ap()

    def window_slice(tab, wv):
        lo = wv * WINDOW
        hi = min(lo + 32768, NCORES * TROWS)
        return tab[lo:hi, :]

    with tile.TileContext(nc, trace_sim=False) as tc:
        with tc.tile_pool(name="const", bufs=1) as cpool, \
             tc.tile_pool(name="gp", bufs=3) as gpool, \
             tc.tile_pool(name="op", bufs=4) as opool, \
             tc.tile_pool(name="ip", bufs=4) as ipool, \
             tc.tile_pool(name="cb", bufs=4) as cbpool, \
             tc.tile_pool(name="ps", bufs=2, space="PSUM") as pspool, \
             tc.tile_pool(name="psacc", bufs=1, space="PSUM") as psacc:

            ident = cpool.tile([128, 128], f32)
            make_identity(nc, ident[:])
            t_dinvf = cpool.tile([128, NCHUNK], f32)
            nc.sync.dma_start(t_dinvf[:], d_dinvf[:])
            t_dinvr = cpool.tile([128, NCHUNK], f32)
            nc.sync.dma_start(t_dinvr[:], d_dinvr[:])
            t_w = {}
            for nm, shp in (("w1f", [IN_C, HID]), ("w1r", [IN_C, HID]),
                            ("w2f", [HID, HID]), ("w2r", [HID, HID]),
                            ("w3f", [HID, OUT_C]), ("w3r", [HID, OUT_C])):
                t_w[nm] = cpool.tile(shp, f32, tag=nm, name=nm)
                nc.sync.dma_start(t_w[nm][:], d_w[nm][:])
            t_b1 = cpool.tile([128, HID], f32)
            nc.sync.dma_start(t_b1[:], d_b1[:])
            t_b2 = cpool.tile([128, HID], f32)
            nc.sync.dma_start(t_b2[:], d_b2[:])
            t_xT = cpool.tile([IN_C, SHARD_PAD], f32)
            nc.sync.dma_start(t_xT[:], d_xT[:])
            t_zero = cpool.tile([1, HID], f32)
            nc.vector.memset(t_zero[:], 0.0)

            def build_table_l1():
                # g1_d = dinv_d * (x @ W1_d); write shard + zero row; AG
                for d in ("f", "r"):
                    wtile = t_w["w1" + d]
                    dv = t_dinvf if d == "f" else t_dinvr
                    for c in range(NCHUNK):
                        ps = pspool.tile([128, HID], f32, tag="bld", space="PSUM")
                        nc.tensor.matmul(ps[:], t_xT[:, c * 128:(c + 1) * 128],
                                         wtile[:], start=True, stop=True)
                        gt = cbpool.tile([128, HID], f32, tag="gout")
                        nc.vector.tensor_scalar_mul(gt[:], ps[:], dv[:, c:c + 1])
                        nrow = min(128, SHARD - c * 128)
                        if nrow > 0:
                            nc.sync.dma_start(d_sh[(1, d)][c * 128:c * 128 + nrow, :],
                                              gt[:nrow, :])
                    nc.sync.dma_start(d_sh[(1, d)][SHARD:SHARD + 1, :], t_zero[:])
                for d in ("f", "r"):
                    nc.gpsimd.collective_compute(
                        "AllGather", mybir.AluOpType.bypass,
                        replica_groups=[list(range(NCORES))],
                        ins=[d_sh[(1, d)][:]], outs=[d_tab[(1, d)][:]])

            def edge_phase(l):
                for d, com, d_gi, d_si in (("f", com_f, d_gif, d_sif),
                                           ("r", com_r, d_gir, d_sir)):
                    tabap = d_tab[(l, d)]
                    aggap = d_agg[(l, d)]
                    for (wv, kk, coff, nrows, roff) in com["calls"]:
                        nch = nrows * kk
                        npos = nch * 128
                        git = ipool.tile([128, CALL_CHUNKS * 8], i16, tag="gi")
                        nc.sync.dma_start(git[:, :npos // 16],
                                          d_gi[:, coff * 8:coff * 8 + npos // 16])
                        gt = gpool.tile([128, CALL_CHUNKS * HID], f32, tag="g")
                        nc.gpsimd.dma_gather(
                            gt[:].rearrange("p (c f) -> p c f", f=HID)[:, :nch, :],
                            window_slice(tabap, wv),
                            git[:, :npos // 16],
                            num_idxs=npos, num_idxs_reg=npos,
                            elem_size=HID, single_packet=False)
                        sit = ipool.tile([128, CALL_CHUNKS * 8], i16, tag="si")
                        nc.sync.dma_start(sit[:, :nrows * 8],
                                          d_si[:, roff * 8:(roff + nrows) * 8])
                        if kk == 1:
                            srcap = gt[:].rearrange("p (c f) -> p c f", f=HID)[:, :nrows, :]
                        else:
                            ot = opool.tile([128, (CALL_CHUNKS // 2) * HID], f32, tag="o")
                            nc.vector.reduce_sum(
                                ot[:].rearrange("p (r f) -> p r f", f=HID)[:, :nrows, :],
                                gt[:].rearrange("p (r k f) -> p r f k", k=kk, f=HID)[:, :nrows, :, :],
                                axis=mybir.AxisListType.X)
                            srcap = ot[:].rearrange("p (r f) -> p r f", f=HID)[:, :nrows, :]
                        nc.gpsimd.dma_scatter_add(
                            aggap[:], srcap, sit[:, :nrows * 8],
                            num_idxs=nrows * 128, num_idxs_reg=nrows * 128,
                            elem_size=HID, single_packet=False)

            def combine_and_build(l):
                # h_l = relu(dinvf*aggf + dinvr*aggr + bias); build tables l+1
                btile = t_b1 if l == 1 else t_b2
                for c in range(NCHUNK):
                    af = cbpool.tile([128, HID], f32, tag="af")
                    nc.sync.dma_start(af[:].rearrange("p (a f) -> p a f", a=1),
                                      d_agg[(l, "f")][c * 128:(c + 1) * 128, :].rearrange("(a p) f -> p a f", p=128))
                    ar = cbpool.tile([128, HID], f32, tag="ar")
                    nc.sync.dma_start(ar[:].rearrange("p (a f) -> p a f", a=1),
                                      d_agg[(l, "r")][c * 128:(c + 1) * 128, :].rearrange("(a p) f -> p a f", p=128))
                    uf = cbpool.tile([128, HID], f32, tag="uf")
                    nc.vector.tensor_scalar_mul(uf[:], af[:], t_dinvf[:, c:c + 1])
                    ur = cbpool.tile([128, HID], f32, tag="ur")
                    nc.vector.tensor_scalar_mul(ur[:], ar[:], t_dinvr[:, c:c + 1])
                    nc.vector.tensor_add(uf[:], uf[:], ur[:])
                    nc.vector.tensor_add(uf[:], uf[:], btile[:])
                    ht = cbpool.tile([128, HID], f32, tag="h")
                    nc.scalar.activation(ht[:], uf[:], mybir.ActivationFunctionType.Relu)
                    nrow = min(128, SHARD - c * 128)
                    if l == 1:
                        # next tables need h^T: transpose, then matmul with W2
                        psT = pspool.tile([128, 128], f32, tag="tr", space="PSUM")
                        nc.tensor.transpose(psT[:], ht[:], ident[:])
                        hT = cbpool.tile([128, HID], f32, tag="hT")
                        nc.scalar.copy(hT[:], psT[:])
                        for d in ("f", "r"):
                            ps2 = pspool.tile([128, HID], f32, tag="bld", space="PSUM")
                            nc.tensor.matmul(ps2[:], hT[:], t_w["w2" + d][:],
                                             start=True, stop=True)
                            gt = cbpool.tile([128, HID], f32, tag="gout")
                            dv = t_dinvf if d == "f" else t_dinvr
                            nc.vector.tensor_scalar_mul(gt[:], ps2[:], dv[:, c:c + 1])
                            if nrow > 0:
                                nc.sync.dma_start(d_sh[(2, d)][c * 128:c * 128 + nrow, :],
                                                  gt[:nrow, :])
                    else:
                        # l == 2 -> tables for l=3 are dinv_d * h2 (no matmul)
                        for d in ("f", "r"):
                            gt = cbpool.tile([128, HID], f32, tag="gout")
                            dv = t_dinvf if d == "f" else t_dinvr
                            nc.vector.tensor_scalar_mul(gt[:], ht[:], dv[:, c:c + 1])
                            if nrow > 0:
                                nc.sync.dma_start(d_sh[(l + 1, d)][c * 128:c * 128 + nrow, :],
                                                  gt[:nrow, :])
                for d in ("f", "r"):
                    nc.sync.dma_start(d_sh[(l + 1, d)][SHARD:SHARD + 1, :], t_zero[:])
                    nc.gpsimd.collective_compute(
                        "AllGather", mybir.AluOpType.bypass,
                        replica_groups=[list(range(NCORES))],
                        ins=[d_sh[(l + 1, d)][:]], outs=[d_tab[(l + 1, d)][:]])

            def pooling_phase():
                t_gm = cpool.tile([128, NCHUNK * GSLOT], f32)
                nc.sync.dma_start(t_gm[:], d_gmat[:])
                pooled = {}
                for d in ("f", "r"):
                    psp = psacc.tile([128, GSLOT], f32, tag="pool" + d, space="PSUM")
                    dv = t_dinvf if d == "f" else t_dinvr
                    for c in range(NCHUNK):
                        az = cbpool.tile([128, HID], f32, tag="af")
                        nc.sync.dma_start(az[:].rearrange("p (a f) -> p a f", a=1),
                                          d_agg[(3, d)][c * 128:(c + 1) * 128, :].rearrange("(a p) f -> p a f", p=128))
                        zt = cbpool.tile([128, HID], f32, tag="z")
                        nc.vector.tensor_scalar_mul(zt[:], az[:], dv[:, c:c + 1])
                        nc.tensor.matmul(psp[:], zt[:],
                                         t_gm[:, c * GSLOT:(c + 1) * GSLOT],
                                         start=(c == 0), stop=(c == NCHUNK - 1))
                    pt = cbpool.tile([128, GSLOT], f32, tag="pooled")
                    nc.scalar.copy(pt[:], psp[:])
                    pooled[d] = pt
                psf = psacc.tile([OUT_C, GSLOT], f32, tag="fin", space="PSUM")
                nc.tensor.matmul(psf[:], t_w["w3f"][:], pooled["f"][:], start=True, stop=False)
                nc.tensor.matmul(psf[:], t_w["w3r"][:], pooled["r"][:], start=False, stop=True)
                ot = cbpool.tile([OUT_C, GSLOT], f32, tag="outt")
                nc.scalar.copy(ot[:], psf[:])
                nc.sync.dma_start(d_out[:], ot[:])

            with nc.named_scope("tab1"):
                build_table_l1()
            with nc.named_scope("edge1"):
                edge_phase(1)
            with nc.named_scope("comb1"):
                combine_and_build(1)
            with nc.named_scope("edge2"):
                edge_phase(2)
            with nc.named_scope("comb2"):
                combine_and_build(2)
            with nc.named_scope("edge3"):
                edge_phase(3)
            with nc.named_scope("pool"):
                pooling_phase()

    nc.compile()

    import os as _os
    res = bass_utils.run_bass_kernel_spmd(nc, in_maps, core_ids=list(range(NCORES)), tmpdir=_os.environ.get('KTMP'))
    LAST["res"] = res

    # ---------- host postprocess ----------
    out_full = np.zeros((G + GSLOT, OUT_C), np.float64)
    for cc in range(NCORES):
        part = res.results[cc]["outp"]          # [OUT_C, GSLOT]
        out_full[g0s[cc]:g0s[cc] + GSLOT, :] += part.T
    cnt = np.bincount(seg, minlength=G).astype(np.float64)
    bias3 = halfa * np.asarray(b_out, np.float64) + halfb * np.asarray(b_out_r, np.float64)
    out = out_full[:G] / np.maximum(cnt, 1.0)[:, None] + bias3[None, :]
    return out.astype(np.float32)


# revision 9
# speedup vs baseline: 1.1308x; 1.1308x over previous
"""DirGNN (3x DirGNNConv + global mean pool) on 8 Trainium2 NeuronCores.

Strategy
--------
Node-sharded data parallelism: core k owns nodes [12500k, 12500(k+1)).
Each DirGNNConv layer l and direction d is factored as

    table_{l,d} = dinv_d * (h_{l-1} @ (alpha_d * W_{l,d}))          # per-node, dense
    agg_{l,d}[head] = sum_{edges e with head(e)=head} table_{l,d}[tail(e)]
    h_l = relu(dinv_f*agg_f + dinv_r*agg_r + 0.5*(b_f + b_r))

so all per-edge work is gather + segment-sum.  Tables are built per-shard and
AllGathered (f32) between layers.  Gathers use the MoE dma_gather DMA
(int16 indices -> table is laid out as 8 x [12500-node shard + zero row] so
every 32K-row index window contains a zero row for padding).  Heads are
decomposed into power-of-two parts (per index-window) so segment sums become
uniform strided DVE reduces; partial sums are combined by dma_scatter_add
(CCE f32 accumulate) into node-ordered aggregate buffers.  The final layer
pools via a PE matmul with a host-built one-hot graph-membership matrix and
applies W_out to the pooled [*,128] vectors; the host sums the 8 partial
[120, 64] outputs, divides by graph sizes and adds biases.

The Bass program is identical on all 8 cores (SPMD); per-core structure is
padded to a common per-(window, part-size) region grid, with padding slots
gathering a zero table row and scattering into a trash row.
"""

import sys

sys.path.insert(0, "/opt/trn_rl_repo")

import numpy as np

N, E, G = 100000, 1600000, 256
IN_C, HID, OUT_C = 4, 128, 120
ALPHA = 0.5
NCORES = 8
SHARD = N // NCORES            # 12500
SHARD_PAD = 12544              # 98 * 128
NCHUNK = SHARD_PAD // 128      # 98
TROWS = SHARD + 1              # table shard rows (12500 + zero row)
# two signed-int16 windows: window w covers table rows < / >= WBOUND and is
# addressed as row = WBASE[w] + idx with idx in [-32768, 32766].  Bases sit
# one row after a zero row so idx == -1 (skipped by ucode) is never needed.
WBOUND = 45268
WBASE = (12501, 75006)
WPAD = (37502 - 12501, 87506 - 75006)   # per-window zero-row idx for padding
NWIN = 2
AGG_ROWS = SHARD_PAD + 1       # 12545, trash row at 12544
TRASH = SHARD_PAD              # 12544
GSLOT = 64                     # max graphs per core in pooled output
KSET = (32, 24, 16, 12, 8, 7, 6, 5, 4, 3, 2, 1)
KLUT = [0] * 256
for _c in range(1, 256):
    for _k in KSET:
        if _k <= _c:
            KLUT[_c] = _k
            break
KLUT = __import__("numpy").array(KLUT)
LAST = {}
CALL_CHUNKS = 32               # gather call size: 32*128 rows


def _table_row(n):
    return n + n // SHARD


def _build_direction(heads, tails):
    """Vectorized edge layout for one direction.

    Returns (common, per_core) where common = list of regions
    (w, k, rows) in program order plus derived call list, and per_core =
    dict core -> (gather_idx int16 [L], scatter_idx int16 [R*128]).
    """
    E_ = heads.shape[0]
    trow = _table_row(tails)
    w = (trow >= WBOUND).astype(np.int64)
    gval = trow - np.array(WBASE)[w]
    core = heads // SHARD
    hloc = heads % SHARD

    korder_of_k = {k: i for i, k in enumerate(KSET)}

    # group edges by (head, window); rank within group
    key = heads.astype(np.int64) * NWIN + w
    order0 = np.argsort(key, kind="stable")
    ks = key[order0]
    grp_start = np.r_[True, ks[1:] != ks[:-1]]
    grp_id = np.cumsum(grp_start) - 1
    first_pos = np.flatnonzero(grp_start)
    r = np.arange(E_) - first_pos[grp_id]
    gsz = np.bincount(grp_id)
    c = gsz[grp_id]

    # greedy largest-part decomposition via KLUT
    k_arr = np.zeros(E_, np.int64)
    j_arr = np.zeros(E_, np.int64)
    part = np.zeros(E_, np.int64)
    cum = np.zeros(E_, np.int64)
    assigned = np.zeros(E_, bool)
    slot = 0
    while not assigned.all():
        ksl = KLUT[np.clip(c - cum, 0, 255)]
        m = (~assigned) & (r < cum + ksl)
        k_arr[m] = ksl[m]
        j_arr[m] = r[m] - cum[m]
        part[m] = slot
        assigned |= m
        cum = cum + ksl
        slot += 1
        assert slot < 64
    ko = np.select([k_arr == kk for kk in KSET], list(range(len(KSET))))

    # order edges by (core, w, ko, hloc, part, j) -- back-to-front keys
    eo = np.lexsort((j_arr, part, hloc[order0], ko, w[order0], core[order0]))
    so = order0[eo]  # original edge ids in final order
    core_s = core[so]
    w_s = w[so]
    ko_s = ko[eo]
    k_s = k_arr[eo]
    j_s = j_arr[eo]
    hloc_s = hloc[so]
    part_s = part[eo]
    gval_s = gval[so]

    # vdst boundaries in the sorted stream
    vkey = np.stack([core_s, w_s, ko_s, hloc_s, part_s])
    vchg = np.r_[True, (np.diff(vkey, axis=1) != 0).any(axis=0)]
    vid = np.cumsum(vchg) - 1                      # global vdst id
    n_vd = vid[-1] + 1
    vd_core = core_s[vchg]
    vd_w = w_s[vchg]
    vd_ko = ko_s[vchg]
    vd_hloc = hloc_s[vchg]

    # vdst counts per (core, w, ko)
    reg_of_vd = (vd_core * NWIN + vd_w) * len(KSET) + vd_ko
    nreg = NCORES * NWIN * len(KSET)
    vd_cnt = np.bincount(reg_of_vd, minlength=nreg).reshape(NCORES, NWIN, len(KSET))
    # common grid: rows per (w, ko) = ceil(max over cores / 128)
    rows_wk = (vd_cnt.max(axis=0) + 127) // 128    # [NWIN, len(KSET)]

    # region order: (w, ko); region chunk offsets
    regions = []   # (w, k, rows, chunk_off, row_off)
    ch_off = 0
    row_off = 0
    for wv in range(NWIN):
        for koi, kk in enumerate(KSET):
            rr = int(rows_wk[wv, koi])
            if rr == 0:
                continue
            regions.append((wv, kk, rr, ch_off, row_off))
            ch_off += rr * kk
            row_off += rr
    L = ch_off * 128          # total gather positions per core
    R = row_off               # total vdst rows per core

    # per-region chunk/row offset lookup tables
    chunk_off_wk = np.zeros((NWIN, len(KSET)), np.int64)
    row_off_wk = np.zeros((NWIN, len(KSET)), np.int64)
    for (wv, kk, rr, co, ro) in regions:
        chunk_off_wk[wv, korder_of_k[kk]] = co
        row_off_wk[wv, korder_of_k[kk]] = ro

    # vdst local index within its (core, region)
    reg_sorted = np.argsort(reg_of_vd, kind="stable")
    vd_local = np.empty(n_vd, np.int64)
    cnts = np.bincount(reg_of_vd, minlength=nreg)
    starts = np.concatenate([[0], np.cumsum(cnts)[:-1]])
    vd_local[reg_sorted] = np.arange(n_vd) - starts[reg_of_vd[reg_sorted]]

    per_core = []
    vd_local_of_edge = vd_local[vid]
    edge_pos = (
        (chunk_off_wk[w_s, ko_s] + (vd_local_of_edge // 128) * k_s + j_s) * 128
        + (vd_local_of_edge % 128)
    )
    vd_rowslot = row_off_wk[vd_w, vd_ko] * 128 + (vd_local // 128) * 128 + (vd_local % 128)

    for cc in range(NCORES):
        gidx = np.empty(L, np.int16)
        for (wv, kk, rr, co, ro) in regions:
            gidx[co * 128:(co + rr * kk) * 128] = WPAD[wv]
        m = core_s == cc
        gidx[edge_pos[m]] = gval_s[m]
        sidx = np.full(R * 128, TRASH, np.int16)
        mv = vd_core == cc
        sidx[vd_rowslot[mv]] = vd_hloc[mv]
        per_core.append((gidx, sidx))

    # call list (common): per region, rows per call = max(1, 32 // k)
    calls = []   # (w, k, chunk_off, rows, row_off)
    for (wv, kk, rr, co, ro) in regions:
        rpc = max(1, CALL_CHUNKS // kk)
        s = 0
        while s < rr:
            nr = min(rpc, rr - s)
            calls.append((wv, kk, co + s * kk, nr, ro + s))
            s += nr

    common = dict(regions=regions, calls=calls, L=L, R=R)
    return common, per_core


def _wrap_idx(a):
    """int16 [M] -> [128, M//16] wrapped (pos i = arr[i%16, i//16]) and
    replicated across the 8 Q7 core partition groups."""
    a16 = a.reshape(-1, 16).T
    return np.ascontiguousarray(np.tile(a16, (8, 1)), dtype=np.int16)


def kernel(x, edge_index, batch_seg, W_in, b_in, W_in_r, b_in_r,
           W_mid, b_mid, W_mid_r, b_mid_r, W_out, b_out, W_out_r, b_out_r):
    import concourse.bass as bass
    import concourse.bacc as bacc
    import concourse.mybir as mybir
    import concourse.tile as tile
    from concourse import bass_utils
    from concourse.masks import make_identity

    x = np.asarray(x, np.float32)
    ei = np.asarray(edge_index, np.int64)
    seg = np.asarray(batch_seg, np.int64)
    Wf1, Wr1 = np.asarray(W_in, np.float32), np.asarray(W_in_r, np.float32)
    Wf2, Wr2 = np.asarray(W_mid, np.float32), np.asarray(W_mid_r, np.float32)
    Wf3, Wr3 = np.asarray(W_out, np.float32), np.asarray(W_out_r, np.float32)
    src, dst = ei[0], ei[1]

    # degree-based norms (match reference: rsqrt(max(deg,1e-12)), 0 for deg==0)
    deg_f = np.bincount(dst, minlength=N).astype(np.float32)
    deg_r = np.bincount(src, minlength=N).astype(np.float32)
    dinv_f = np.where(deg_f > 0, 1.0 / np.sqrt(np.maximum(deg_f, 1e-12)), 0.0).astype(np.float32)
    dinv_r = np.where(deg_r > 0, 1.0 / np.sqrt(np.maximum(deg_r, 1e-12)), 0.0).astype(np.float32)

    com_f, pc_f = _build_direction(dst, src)     # fwd: head=dst, tail=src
    com_r, pc_r = _build_direction(src, dst)     # rev: head=src, tail=dst

    # ---------- per-core host tensors ----------
    halfa = np.float32(1.0 - ALPHA)   # fwd scale
    halfb = np.float32(ALPHA)         # rev scale
    W1f_s = (Wf1 * halfa).astype(np.float32)         # [4,128]
    W1r_s = (Wr1 * halfb).astype(np.float32)
    W2f_s = (Wf2 * halfa).astype(np.float32)         # [128,128]
    W2r_s = (Wr2 * halfb).astype(np.float32)
    W3f_s = (Wf3 * halfa).astype(np.float32)         # [128,120]
    W3r_s = (Wr3 * halfb).astype(np.float32)
    bias1 = (halfa * np.asarray(b_in, np.float32) + halfb * np.asarray(b_in_r, np.float32))
    bias2 = (halfa * np.asarray(b_mid, np.float32) + halfb * np.asarray(b_mid_r, np.float32))

    xfull_pad = np.zeros((NCORES, SHARD_PAD, IN_C), np.float32)
    seg_pad = np.zeros((NCORES, SHARD_PAD), np.int64)
    dinvf_pad = np.zeros((NCORES, SHARD_PAD), np.float32)
    dinvr_pad = np.zeros((NCORES, SHARD_PAD), np.float32)
    for cc in range(NCORES):
        xfull_pad[cc, :SHARD] = x[cc * SHARD:(cc + 1) * SHARD]
        seg_pad[cc, :SHARD] = seg[cc * SHARD:(cc + 1) * SHARD]
        dinvf_pad[cc, :SHARD] = dinv_f[cc * SHARD:(cc + 1) * SHARD]
        dinvr_pad[cc, :SHARD] = dinv_r[cc * SHARD:(cc + 1) * SHARD]

    g0s = [int(seg[cc * SHARD]) for cc in range(NCORES)]
    for cc in range(NCORES):
        gmax = int(seg_pad[cc, :SHARD].max())
        assert gmax - g0s[cc] < GSLOT, "graph range exceeds GSLOT"

    LF, RF = com_f["L"], com_f["R"]
    LR, RR = com_r["L"], com_r["R"]

    in_maps = []
    for cc in range(NCORES):
        gi_f, si_f = pc_f[cc]
        gi_r, si_r = pc_r[cc]
        # p-major [128, NCHUNK] node-chunk tiles
        dvf_t = dinvf_pad[cc].reshape(NCHUNK, 128).T.copy()
        dvr_t = dinvr_pad[cc].reshape(NCHUNK, 128).T.copy()
        # graph one-hot [128, NCHUNK*GSLOT]
        gm = np.zeros((128, NCHUNK, GSLOT), np.float32)
        locg = seg_pad[cc] - g0s[cc]
        pp = np.arange(SHARD_PAD) % 128
        ch = np.arange(SHARD_PAD) // 128
        real = np.arange(SHARD_PAD) < SHARD
        gm[pp[real], ch[real], locg[real]] = 1.0
        im = {
            "xT": np.ascontiguousarray(xfull_pad[cc].T),          # [4, SHARD_PAD]
            "dinvf": dvf_t, "dinvr": dvr_t,
            "gmat": gm.reshape(128, NCHUNK * GSLOT),
            "w1f": W1f_s, "w1r": W1r_s,
            "w2f": W2f_s, "w2r": W2r_s,
            "w3f": W3f_s, "w3r": W3r_s,
            "bias1": np.tile(bias1.reshape(1, HID), (128, 1)), "bias2": np.tile(bias2.reshape(1, HID), (128, 1)),
            "gif": _wrap_idx(gi_f), "gir": _wrap_idx(gi_r),
            "sif": _wrap_idx(si_f), "sir": _wrap_idx(si_r),
        }
        for l in (1, 2, 3):
            for d in ("f", "r"):
                im[f"agg{l}{d}"] = np.zeros((AGG_ROWS, HID), np.float32)
        in_maps.append(im)

    # ---------- bass program (identical on all cores) ----------
    nc = bacc.Bacc("TRN2", target_bir_lowering=False, debug=False,
                   enable_asserts=False, num_devices=NCORES)
    f32 = mybir.dt.float32
    i16 = mybir.dt.int16

    d_xT = nc.dram_tensor("xT", [IN_C, SHARD_PAD], f32, kind="ExternalInput").ap()
    d_dinvf = nc.dram_tensor("dinvf", [128, NCHUNK], f32, kind="ExternalInput").ap()
    d_dinvr = nc.dram_tensor("dinvr", [128, NCHUNK], f32, kind="ExternalInput").ap()
    d_gmat = nc.dram_tensor("gmat", [128, NCHUNK * GSLOT], f32, kind="ExternalInput").ap()
    d_w = {}
    d_w["w1f"] = nc.dram_tensor("w1f", [IN_C, HID], f32, kind="ExternalInput").ap()
    d_w["w1r"] = nc.dram_tensor("w1r", [IN_C, HID], f32, kind="ExternalInput").ap()
    d_w["w2f"] = nc.dram_tensor("w2f", [HID, HID], f32, kind="ExternalInput").ap()
    d_w["w2r"] = nc.dram_tensor("w2r", [HID, HID], f32, kind="ExternalInput").ap()
    d_w["w3f"] = nc.dram_tensor("w3f", [HID, OUT_C], f32, kind="ExternalInput").ap()
    d_w["w3r"] = nc.dram_tensor("w3r", [HID, OUT_C], f32, kind="ExternalInput").ap()
    d_b1 = nc.dram_tensor("bias1", [128, HID], f32, kind="ExternalInput").ap()
    d_b2 = nc.dram_tensor("bias2", [128, HID], f32, kind="ExternalInput").ap()
    d_gif = nc.dram_tensor("gif", [128, LF // 16], i16, kind="ExternalInput").ap()
    d_gir = nc.dram_tensor("gir", [128, LR // 16], i16, kind="ExternalInput").ap()
    d_sif = nc.dram_tensor("sif", [128, RF * 8], i16, kind="ExternalInput").ap()
    d_sir = nc.dram_tensor("sir", [128, RR * 8], i16, kind="ExternalInput").ap()
    d_agg = {}
    for l in (1, 2, 3):
        for d in ("f", "r"):
            d_agg[(l, d)] = nc.dram_tensor(f"agg{l}{d}", [AGG_ROWS, HID], f32,
                                           kind="ExternalInput").ap()
    d_out = nc.dram_tensor("outp", [OUT_C, GSLOT], f32, kind="ExternalOutput").ap()

    # internal HBM: table shards + gathered tables
    d_sh = {}
    d_tab = {}
    for l in (1, 2, 3):
        for d in ("f", "r"):
            d_sh[(l, d)] = nc.dram_tensor(f"sh{l}{d}", [TROWS, HID], f32, kind="Internal").ap()
            d_tab[(l, d)] = nc.dram_tensor(f"tab{l}{d}", [NCORES * TROWS, HID], f32,
                                           kind="Internal", addr_space="Shared").ap()

    def window_slice(tab, wv):
        lo = WBASE[wv]
        hi = min(lo + 32767, NCORES * TROWS)
        return tab[lo:hi, :]

    with tile.TileContext(nc, trace_sim=False) as tc:
        with tc.tile_pool(name="const", bufs=1) as cpool, \
             tc.tile_pool(name="gp", bufs=3) as gpool, \
             tc.tile_pool(name="op", bufs=4) as opool, \
             tc.tile_pool(name="ip", bufs=4) as ipool, \
             tc.tile_pool(name="cb", bufs=4) as cbpool, \
             tc.tile_pool(name="ps", bufs=2, space="PSUM") as pspool, \
             tc.tile_pool(name="psacc", bufs=1, space="PSUM") as psacc:

            ident = cpool.tile([128, 128], f32)
            make_identity(nc, ident[:])
            t_dinvf = cpool.tile([128, NCHUNK], f32)
            nc.sync.dma_start(t_dinvf[:], d_dinvf[:])
            t_dinvr = cpool.tile([128, NCHUNK], f32)
            nc.sync.dma_start(t_dinvr[:], d_dinvr[:])
            t_w = {}
            for nm, shp in (("w1f", [IN_C, HID]), ("w1r", [IN_C, HID]),
                            ("w2f", [HID, HID]), ("w2r", [HID, HID]),
                            ("w3f", [HID, OUT_C]), ("w3r", [HID, OUT_C])):
                t_w[nm] = cpool.tile(shp, f32, tag=nm, name=nm)
                nc.sync.dma_start(t_w[nm][:], d_w[nm][:])
            t_b1 = cpool.tile([128, HID], f32)
            nc.sync.dma_start(t_b1[:], d_b1[:])
            t_b2 = cpool.tile([128, HID], f32)
            nc.sync.dma_start(t_b2[:], d_b2[:])
            t_xT = cpool.tile([IN_C, SHARD_PAD], f32)
            nc.sync.dma_start(t_xT[:], d_xT[:])
            t_zero = cpool.tile([1, HID], f32)
            nc.vector.memset(t_zero[:], 0.0)

            def build_table_l1():
                # g1_d = dinv_d * (x @ W1_d); write shard + zero row; AG
                for d in ("f", "r"):
                    wtile = t_w["w1" + d]
                    dv = t_dinvf if d == "f" else t_dinvr
                    for c in range(NCHUNK):
                        ps = pspool.tile([128, HID], f32, tag="bld", space="PSUM")
                        nc.tensor.matmul(ps[:], t_xT[:, c * 128:(c + 1) * 128],
                                         wtile[:], start=True, stop=True)
                        gt = cbpool.tile([128, HID], f32, tag="gout")
                        nc.vector.tensor_scalar_mul(gt[:], ps[:], dv[:, c:c + 1])
                        nrow = min(128, SHARD - c * 128)
                        if nrow > 0:
                            nc.sync.dma_start(d_sh[(1, d)][c * 128:c * 128 + nrow, :],
                                              gt[:nrow, :])
                    nc.sync.dma_start(d_sh[(1, d)][SHARD:SHARD + 1, :], t_zero[:])
                for d in ("f", "r"):
                    nc.gpsimd.collective_compute(
                        "AllGather", mybir.AluOpType.bypass,
                        replica_groups=[list(range(NCORES))],
                        ins=[d_sh[(1, d)][:]], outs=[d_tab[(1, d)][:]])

            def edge_phase(l):
                for d, com, d_gi, d_si in (("f", com_f, d_gif, d_sif),
                                           ("r", com_r, d_gir, d_sir)):
                    tabap = d_tab[(l, d)]
                    aggap = d_agg[(l, d)]
                    for (wv, kk, coff, nrows, roff) in com["calls"]:
                        nch = nrows * kk
                        npos = nch * 128
                        git = ipool.tile([128, CALL_CHUNKS * 8], i16, tag="gi")
                        nc.sync.dma_start(git[:, :npos // 16],
                                          d_gi[:, coff * 8:coff * 8 + npos // 16])
                        gt = gpool.tile([128, CALL_CHUNKS * HID], f32, tag="g")
                        nc.gpsimd.dma_gather(
                            gt[:, :nch * HID].rearrange("p (c f) -> p c f", f=HID),
                            window_slice(tabap, wv),
                            git[:, :npos // 16],
                            num_idxs=npos, num_idxs_reg=npos,
                            elem_size=HID, single_packet=False)
                        sit = ipool.tile([128, CALL_CHUNKS * 8], i16, tag="si")
                        nc.sync.dma_start(sit[:, :nrows * 8],
                                          d_si[:, roff * 8:(roff + nrows) * 8])
                        if kk == 1:
                            srcap = gt[:, :nrows * HID].rearrange("p (c f) -> p c f", f=HID)
                        else:
                            ot = opool.tile([128, (CALL_CHUNKS // 2) * HID], f32, tag="o")
                            nc.vector.reduce_sum(
                                ot[:, :nrows * HID].rearrange("p (r f) -> p r f", f=HID),
                                gt[:, :nrows * kk * HID].rearrange("p (r k f) -> p r f k", k=kk, f=HID),
                                axis=mybir.AxisListType.X)
                            srcap = ot[:, :nrows * HID].rearrange("p (r f) -> p r f", f=HID)
                        nc.gpsimd.dma_scatter_add(
                            aggap[:], srcap, sit[:, :nrows * 8],
                            num_idxs=nrows * 128, num_idxs_reg=nrows * 128,
                            elem_size=HID, single_packet=False)

            def combine_and_build(l):
                # h_l = relu(dinvf*aggf + dinvr*aggr + bias); build tables l+1
                btile = t_b1 if l == 1 else t_b2
                for c in range(NCHUNK):
                    af = cbpool.tile([128, HID], f32, tag="af")
                    nc.sync.dma_start(af[:].rearrange("p (a f) -> p a f", a=1),
                                      d_agg[(l, "f")][c * 128:(c + 1) * 128, :].rearrange("(a p) f -> p a f", p=128))
                    ar = cbpool.tile([128, HID], f32, tag="ar")
                    nc.sync.dma_start(ar[:].rearrange("p (a f) -> p a f", a=1),
                                      d_agg[(l, "r")][c * 128:(c + 1) * 128, :].rearrange("(a p) f -> p a f", p=128))
                    uf = cbpool.tile([128, HID], f32, tag="uf")
                    nc.vector.tensor_scalar_mul(uf[:], af[:], t_dinvf[:, c:c + 1])
                    ur = cbpool.tile([128, HID], f32, tag="ur")
                    nc.vector.tensor_scalar_mul(ur[:], ar[:], t_dinvr[:, c:c + 1])
                    nc.vector.tensor_add(uf[:], uf[:], ur[:])
                    nc.vector.tensor_add(uf[:], uf[:], btile[:])
                    ht = cbpool.tile([128, HID], f32, tag="h")
                    nc.scalar.activation(ht[:], uf[:], mybir.ActivationFunctionType.Relu)
                    nrow = min(128, SHARD - c * 128)
                    if l == 1:
                        # next tables need h^T: transpose, then matmul with W2
                        psT = pspool.tile([128, 128], f32, tag="tr", space="PSUM")
                        nc.tensor.transpose(psT[:], ht[:], ident[:])
                        hT = cbpool.tile([128, HID], f32, tag="hT")
                        nc.scalar.copy(hT[:], psT[:])
                        for d in ("f", "r"):
                            ps2 = pspool.tile([128, HID], f32, tag="bld", space="PSUM")
                            nc.tensor.matmul(ps2[:], hT[:], t_w["w2" + d][:],
                                             start=True, stop=True)
                            gt = cbpool.tile([128, HID], f32, tag="gout")
                            dv = t_dinvf if d == "f" else t_dinvr
                            nc.vector.tensor_scalar_mul(gt[:], ps2[:], dv[:, c:c + 1])
                            if nrow > 0:
                                nc.sync.dma_start(d_sh[(2, d)][c * 128:c * 128 + nrow, :],
                                                  gt[:nrow, :])
                    else:
                        # l == 2 -> tables for l=3 are dinv_d * h2 (no matmul)
                        for d in ("f", "r"):
                            gt = cbpool.tile([128, HID], f32, tag="gout")
                            dv = t_dinvf if d == "f" else t_dinvr
                            nc.vector.tensor_scalar_mul(gt[:], ht[:], dv[:, c:c + 1])
                            if nrow > 0:
                                nc.sync.dma_start(d_sh[(l + 1, d)][c * 128:c * 128 + nrow, :],
                                                  gt[:nrow, :])
                for d in ("f", "r"):
                    nc.sync.dma_start(d_sh[(l + 1, d)][SHARD:SHARD + 1, :], t_zero[:])
                    nc.gpsimd.collective_compute(
                        "AllGather", mybir.AluOpType.bypass,
                        replica_groups=[list(range(NCORES))],
                        ins=[d_sh[(l + 1, d)][:]], outs=[d_tab[(l + 1, d)][:]])

            def pooling_phase():
                t_gm = cpool.tile([128, NCHUNK * GSLOT], f32)
                nc.sync.dma_start(t_gm[:], d_gmat[:])
                pooled = {}
                for d in ("f", "r"):
                    psp = psacc.tile([128, GSLOT], f32, tag="pool" + d, space="PSUM")
                    dv = t_dinvf if d == "f" else t_dinvr
                    for c in range(NCHUNK):
                        az = cbpool.tile([128, HID], f32, tag="af")
                        nc.sync.dma_start(az[:].rearrange("p (a f) -> p a f", a=1),
                                          d_agg[(3, d)][c * 128:(c + 1) * 128, :].rearrange("(a p) f -> p a f", p=128))
                        zt = cbpool.tile([128, HID], f32, tag="z")
                        nc.vector.tensor_scalar_mul(zt[:], az[:], dv[:, c:c + 1])
                        nc.tensor.matmul(psp[:], zt[:],
                                         t_gm[:, c * GSLOT:(c + 1) * GSLOT],
                                         start=(c == 0), stop=(c == NCHUNK - 1))
                    pt = cbpool.tile([128, GSLOT], f32, tag="pooled")
                    nc.scalar.copy(pt[:], psp[:])
                    pooled[d] = pt
                psf = psacc.tile([OUT_C, GSLOT], f32, tag="fin", space="PSUM")
                nc.tensor.matmul(psf[:], t_w["w3f"][:], pooled["f"][:], start=True, stop=False)
                nc.tensor.matmul(psf[:], t_w["w3r"][:], pooled["r"][:], start=False, stop=True)
                ot = cbpool.tile([OUT_C, GSLOT], f32, tag="outt")
                nc.scalar.copy(ot[:], psf[:])
                nc.sync.dma_start(d_out[:], ot[:])

            with nc.named_scope("tab1"):
                build_table_l1()
            with nc.named_scope("edge1"):
                edge_phase(1)
            with nc.named_scope("comb1"):
                combine_and_build(1)
            with nc.named_scope("edge2"):
                edge_phase(2)
            with nc.named_scope("comb2"):
                combine_and_build(2)
            with nc.named_scope("edge3"):
                edge_phase(3)
            with nc.named_scope("pool"):
                pooling_phase()

    nc.compile()

    import os as _os
    res = bass_utils.run_bass_kernel_spmd(nc, in_maps, core_ids=list(range(NCORES)), tmpdir=_os.environ.get('KTMP'))
    LAST["res"] = res

    # ---------- host postprocess ----------
    out_full = np.zeros((G + GSLOT, OUT_C), np.float64)
    for cc in range(NCORES):
        part = res.results[cc]["outp"]          # [OUT_C, GSLOT]
        out_full[g0s[cc]:g0s[cc] + GSLOT, :] += part.T
    cnt = np.bincount(seg, minlength=G).astype(np.float64)
    bias3 = halfa * np.asarray(b_out, np.float64) + halfb * np.asarray(b_out_r, np.float64)
    out = out_full[:G] / np.maximum(cnt, 1.0)[:, None] + bias3[None, :]
    return out.astype(np.float32)
